# revision 10
# baseline (speedup 1.0000x reference)
"""Trainium2 Bass kernel for nn_DLK_35218731827409 (dense_cnn LKA-style block).

Reference computation (per batch, 64 channels, 64^3 volume):
    att1 = depthwise_conv3d(x, w1 5x5x5, pad 2) + b1
    att2 = depthwise_conv3d(att1, w2 7x7x7, dil 3, pad 9) + b2
    avg/max pooling over the 128 channels of concat(att1, att2)
    gate = sigmoid(conv3d(pooled, ws 2->2ch 7x7x7, pad 3) + bs)
    out  = att1*gate0 + att2*gate1 + x

Sharding: channels (64 -> 8 per core, both batches on every core). Depthwise
convs are channel-independent -> no halo, no redundant compute. The
cross-channel mean/max pooling is the only global step: per-core partial
sum/max + AllReduce(add) + AllReduce(max) over the 8 cores. Every core then
computes the (small) gate conv for the full volume and combines its own
channels.

Conv mapping on the TensorEngine: partitions = (2 channels x 64 H rows),
contraction along H via a banded Toeplitz lhsT (built host-side from the
depthwise weights), one PSUM-accumulated float32r matmul (N=512) per
(kd, kw) tap over (D, W) columns.
"""
import sys
import types

import numpy as np


def _install_ntff_hook():
    # Provide the antenv.axon_hooks module this image lacks so that
    # run_bass_kernel_spmd(trace=True) can reach the NTFF profiler
    # (documented degraded path in trn_agent_boot/trn_boot.py).
    if "antenv.axon_hooks" in sys.modules:
        return
    try:
        from trn_agent_boot.trn_boot import _ntff_profile_via_ctypes

        hook = _ntff_profile_via_ctypes("/opt/axon/libaxon_pjrt.so")
    except Exception:
        hook = None
    mod = types.ModuleType("antenv.axon_hooks")
    mod._hook = hook
    mod.get_axon_ntff_profile_hook = lambda: mod._hook
    mod.set_axon_ntff_profile_hook = lambda h: setattr(mod, "_hook", h)
    try:
        import antenv

        sys.modules["antenv.axon_hooks"] = mod
        antenv.axon_hooks = mod
    except Exception:
        pass


_install_ntff_hook()

import concourse.bacc as bacc
import concourse.bass_utils as bass_utils
import concourse.mybir as mybir
import concourse.tile as tile

dt = mybir.dt
AF = mybir.ActivationFunctionType
ALU = mybir.AluOpType

B, C, S = 2, 64, 64
N_CORES = 8
CPC = C // N_CORES  # 8 channels per core
PAIRS = CPC // 2  # 4 channel pairs per core
NEG_INF = -1.0e30


# ---------------------------------------------------------------- host prep
def _build_bands1(w1c):
    """w1c: [8, 1, 5, 5, 5] -> [PAIRS, 128(k), 25(t), 128(m)] f32."""
    out = np.zeros((PAIRS, 128, 25, 128), np.float32)
    ho = np.arange(64)
    for p in range(PAIRS):
        for c2 in range(2):
            c = 2 * p + c2
            for kd in range(5):
                for kw in range(5):
                    for kh in range(5):
                        hi = ho + kh - 2
                        m = (hi >= 0) & (hi < 64)
                        out[p, c2 * 64 + hi[m], kd * 5 + kw, c2 * 64 + ho[m]] = w1c[
                            c, 0, kd, kh, kw
                        ]
    return out


def _build_bands2(w2c):
    out = np.zeros((PAIRS, 128, 49, 128), np.float32)
    ho = np.arange(64)
    for p in range(PAIRS):
        for c2 in range(2):
            c = 2 * p + c2
            for kd in range(7):
                for kw in range(7):
                    for kh in range(7):
                        hi = ho + 3 * kh - 9
                        m = (hi >= 0) & (hi < 64)
                        out[p, c2 * 64 + hi[m], kd * 7 + kw, c2 * 64 + ho[m]] = w2c[
                            c, 0, kd, kh, kw
                        ]
    return out


def _build_bandsg(ws):
    """ws: [2, 2, 7, 7, 7]; fold mean 1/128 into ci=0. -> [128, 49, 128]."""
    wsx = np.array(ws, np.float32).copy()
    wsx[:, 0] /= 128.0
    out = np.zeros((128, 49, 128), np.float32)
    ho = np.arange(64)
    for ci in range(2):
        for co in range(2):
            for kd in range(7):
                for kw in range(7):
                    for kh in range(7):
                        hi = ho + kh - 3
                        m = (hi >= 0) & (hi < 64)
                        out[ci * 64 + hi[m], kd * 7 + kw, co * 64 + ho[m]] = wsx[
                            co, ci, kd, kh, kw
                        ]
    return out


# ---------------------------------------------------------------- program
_CACHE = {}


def _build_program():
    if "nc" in _CACHE:
        return _CACHE["nc"]
    f32, f32r = dt.float32, dt.float32r
    nc = bacc.Bacc(
        "TRN2", target_bir_lowering=False, debug=False, num_devices=N_CORES
    )
    xz = nc.dram_tensor("xz", [B, CPC, S + 4, S, S + 4], f32r, kind="ExternalInput").ap()
    xin = nc.dram_tensor("xin", [B, CPC, S, S, S], f32, kind="ExternalInput").ap()
    bd1_d = nc.dram_tensor("bands1", [PAIRS, 128, 25, 128], f32r, kind="ExternalInput").ap()
    bd2_d = nc.dram_tensor("bands2", [PAIRS, 128, 49, 128], f32r, kind="ExternalInput").ap()
    bdg_d = nc.dram_tensor("bandsg", [128, 49, 128], f32r, kind="ExternalInput").ap()
    b1_d = nc.dram_tensor("bias1", [128, PAIRS], f32, kind="ExternalInput").ap()
    b2_d = nc.dram_tensor("bias2", [128, PAIRS], f32, kind="ExternalInput").ap()
    bg_d = nc.dram_tensor("biasg", [128, 1], f32, kind="ExternalInput").ap()
    zp = nc.dram_tensor("zpad", [128, 64, S + 18], f32r, kind="ExternalInput").ap()
    out_d = nc.dram_tensor("out", [B, CPC, S, S, S], f32, kind="ExternalOutput").ap()

    with tile.TileContext(nc) as tc:
        with (
            tc.tile_pool(name="const", bufs=1) as constp,
            tc.tile_pool(name="dram", bufs=1, space="DRAM") as dram,
        ):
            bias1_t = constp.tile([128, PAIRS], f32)
            bias2_t = constp.tile([128, PAIRS], f32)
            biasg_t = constp.tile([128, 1], f32)
            nc.sync.dma_start(bias1_t[:], b1_d[:])
            nc.sync.dma_start(bias2_t[:], b2_d[:])
            nc.sync.dma_start(biasg_t[:], bg_d[:])

            att1_sp = dram.tile([B, PAIRS, 128, S, S], f32)
            att2_sp = dram.tile([B, PAIRS, 128, S, S], f32)
            ps_in = dram.tile([B, 64, S, S], f32)
            pm_in = dram.tile([B, 64, S, S], f32)
            ps_out = dram.tile([B, 64, S, S], f32)
            pm_out = dram.tile([B, 64, S, S], f32)

            # ---------------- conv phase ----------------
            with (
                tc.tile_pool(name="csb", bufs=1) as csb,
                tc.tile_pool(name="cps", bufs=4, space="PSUM") as cps,
            ):
                for b in range(B):
                    acc_s = csb.tile([128, S, S], f32, tag="acc_s")
                    acc_m = csb.tile([128, S, S], f32, tag="acc_m")
                    nc.vector.memset(acc_s[:], 0.0)
                    nc.vector.memset(acc_m[:], NEG_INF)
                    for p in range(PAIRS):
                        xt = csb.tile([128, S + 4, S + 4], f32r, tag="xt", bufs=2)
                        for c2 in range(2):
                            nc.sync.dma_start(
                                xt[64 * c2 : 64 * c2 + 64],
                                xz[b, 2 * p + c2].transpose([1, 0, 2]),
                            )
                        bd1 = csb.tile([128, 25, 128], f32r, tag="bd1", bufs=2)
                        nc.sync.dma_start(bd1[:], bd1_d[p])
                        bd2 = csb.tile([128, 49, 128], f32r, tag="bd2", bufs=1)
                        nc.sync.dma_start(bd2[:], bd2_d[p])

                        att1 = csb.tile([128, S + 18, S + 18], f32r, tag="att1")
                        # zero the halo border (interior is fully overwritten).
                        # memset lacks f32r ucode, so DMA from a zeros input.
                        nc.sync.dma_start(att1[:, 0:9, :], zp[:, 0:9, :])
                        nc.sync.dma_start(att1[:, S + 9 : S + 18, :], zp[:, 0:9, :])
                        nc.sync.dma_start(att1[:, 9 : S + 9, 0:9], zp[:, :, 0:9])
                        nc.sync.dma_start(
                            att1[:, 9 : S + 9, S + 9 : S + 18], zp[:, :, 0:9]
                        )

                        # conv1: 8 groups x 25 taps
                        for g in range(8):
                            ps = cps.tile([128, 8, 64], f32, tag="ps")
                            for kd in range(5):
                                for kw in range(5):
                                    t = kd * 5 + kw
                                    nc.tensor.matmul(
                                        ps[:],
                                        bd1[:, t, :],
                                        xt[:, 8 * g + kd : 8 * g + kd + 8, kw : kw + 64],
                                        start=(t == 0),
                                        stop=(t == 24),
                                    )
                            nc.scalar.activation(
                                att1[:, 9 + 8 * g : 17 + 8 * g, 9 : S + 9],
                                ps[:],
                                AF.Identity,
                                bias=bias1_t[:, p : p + 1],
                            )
                        # read the f32r interior as plain f32 bits
                        att1_int = att1[:, 9 : S + 9, 9 : S + 9].bitcast(f32)
                        nc.sync.dma_start(att1_sp[b, p], att1_int)
                        nc.vector.tensor_tensor(acc_s[:], acc_s[:], att1_int, ALU.add)
                        nc.vector.tensor_tensor(acc_m[:], acc_m[:], att1_int, ALU.max)

                        att2 = csb.tile([128, S, S], f32, tag="att2")
                        # conv2: 8 groups x 49 taps (dilation 3)
                        for g in range(8):
                            ps2 = cps.tile([128, 8, 64], f32, tag="ps")
                            for kd in range(7):
                                for kw in range(7):
                                    t = kd * 7 + kw
                                    nc.tensor.matmul(
                                        ps2[:],
                                        bd2[:, t, :],
                                        att1[
                                            :,
                                            8 * g + 3 * kd : 8 * g + 3 * kd + 8,
                                            3 * kw : 3 * kw + 64,
                                        ],
                                        start=(t == 0),
                                        stop=(t == 48),
                                    )
                            nc.scalar.activation(
                                att2[:, 8 * g : 8 * g + 8, :],
                                ps2[:],
                                AF.Identity,
                                bias=bias2_t[:, p : p + 1],
                            )
                        nc.sync.dma_start(att2_sp[b, p], att2[:])
                        nc.vector.tensor_tensor(acc_s[:], acc_s[:], att2[:], ALU.add)
                        nc.vector.tensor_tensor(acc_m[:], acc_m[:], att2[:], ALU.max)

                    # fold the two channel halves (cross-partition move via
                    # SBUF->SBUF DMA, then DVE combine) and stage for the
                    # AllReduce.
                    tmp_s = csb.tile([64, S, S], f32, tag="tmp_s")
                    tmp_m = csb.tile([64, S, S], f32, tag="tmp_m")
                    nc.sync.dma_start(tmp_s[:], acc_s[64:128])
                    nc.sync.dma_start(tmp_m[:], acc_m[64:128])
                    nc.vector.tensor_tensor(tmp_s[:], tmp_s[:], acc_s[0:64], ALU.add)
                    nc.vector.tensor_tensor(tmp_m[:], tmp_m[:], acc_m[0:64], ALU.max)
                    nc.sync.dma_start(ps_in[b], tmp_s[:])
                    nc.sync.dma_start(pm_in[b], tmp_m[:])

            # ---------------- cross-core pooling ----------------
            nc.gpsimd.collective_compute(
                "AllReduce",
                ALU.add,
                replica_groups=[list(range(N_CORES))],
                ins=[ps_in.opt()],
                outs=[ps_out.opt()],
            )
            nc.gpsimd.collective_compute(
                "AllReduce",
                ALU.max,
                replica_groups=[list(range(N_CORES))],
                ins=[pm_in.opt()],
                outs=[pm_out.opt()],
            )

            # ---------------- gate + final combine ----------------
            with (
                tc.tile_pool(name="gsb", bufs=1) as gsb,
                tc.tile_pool(name="gps", bufs=4, space="PSUM") as gps,
            ):
                bdg = gsb.tile([128, 49, 128], f32r, tag="bdg")
                nc.sync.dma_start(bdg[:], bdg_d[:])
                for b in range(B):
                    # pooled, in (ci, h) x (d, w) layout with zero borders.
                    # Loaded as f32 then ACT-copied so the f32r-consuming
                    # gate matmul sees an f32r-rounding producer.
                    pt0 = gsb.tile([128, S + 6, S + 6], f32, tag="pt0")
                    nc.vector.memset(pt0[:], 0.0)
                    nc.sync.dma_start(pt0[0:64, 3 : S + 3, 3 : S + 3], ps_out[b])
                    nc.sync.dma_start(pt0[64:128, 3 : S + 3, 3 : S + 3], pm_out[b])
                    pt = gsb.tile([128, S + 6, S + 6], f32r, tag="pt")
                    nc.scalar.activation(pt[:], pt0[:], AF.Copy)

                    gA = gsb.tile([128, S, S], f32, tag="gA")
                    gB = gsb.tile([128, S, S], f32, tag="gB")
                    for g in range(8):
                        psg = gps.tile([128, 8, 64], f32, tag="psg")
                        for kd in range(7):
                            for kw in range(7):
                                t = kd * 7 + kw
                                nc.tensor.matmul(
                                    psg[:],
                                    bdg[:, t, :],
                                    pt[:, 8 * g + kd : 8 * g + kd + 8, kw : kw + 64],
                                    start=(t == 0),
                                    stop=(t == 48),
                                )
                        nc.scalar.activation(
                            gA[0:64, 8 * g : 8 * g + 8, :],
                            psg[0:64],
                            AF.Sigmoid,
                            bias=biasg_t[0:64, 0:1],
                        )
                        nc.scalar.activation(
                            gB[64:128, 8 * g : 8 * g + 8, :],
                            psg[64:128],
                            AF.Sigmoid,
                            bias=biasg_t[64:128, 0:1],
                        )
                    # replicate gate planes to the other channel half
                    # (cross-partition -> SBUF->SBUF DMA)
                    nc.sync.dma_start(gA[64:128], gA[0:64])
                    nc.sync.dma_start(gB[0:64], gB[64:128])

                    for p in range(PAIRS):
                        a1f = gsb.tile([128, S, S], f32, tag="a1f", bufs=2)
                        a2f = gsb.tile([128, S, S], f32, tag="a2f", bufs=2)
                        xf = gsb.tile([128, S, S], f32, tag="xf", bufs=1)
                        nc.sync.dma_start(a1f[:], att1_sp[b, p])
                        nc.sync.dma_start(a2f[:], att2_sp[b, p])
                        for c2 in range(2):
                            nc.sync.dma_start(
                                xf[64 * c2 : 64 * c2 + 64],
                                xin[b, 2 * p + c2].transpose([1, 0, 2]),
                            )
                        ot = gsb.tile([128, S, S], f32, tag="ot", bufs=1)
                        nc.vector.tensor_tensor(a1f[:], a1f[:], gA[:], ALU.mult)
                        nc.vector.tensor_tensor(a2f[:], a2f[:], gB[:], ALU.mult)
                        nc.vector.tensor_tensor(ot[:], a1f[:], a2f[:], ALU.add)
                        nc.vector.tensor_tensor(ot[:], ot[:], xf[:], ALU.add)
                        for c2 in range(2):
                            nc.sync.dma_start(
                                out_d[b, 2 * p + c2].transpose([1, 0, 2]),
                                ot[64 * c2 : 64 * c2 + 64],
                            )

    nc.compile()
    _CACHE["nc"] = nc
    return nc


# ---------------------------------------------------------------- runner
def _prepare_in_maps(x, w1, b1, w2, b2, ws, bs):
    x = np.ascontiguousarray(np.asarray(x, np.float32))
    w1 = np.asarray(w1, np.float32)
    b1 = np.asarray(b1, np.float32)
    w2 = np.asarray(w2, np.float32)
    b2 = np.asarray(b2, np.float32)
    ws = np.asarray(ws, np.float32)
    bs = np.asarray(bs, np.float32)

    bandsg = _build_bandsg(ws)
    biasg = np.repeat(bs, 64).reshape(128, 1).astype(np.float32)
    zpad = np.zeros((128, 64, S + 18), np.float32)

    in_maps = []
    for core in range(N_CORES):
        ch = slice(CPC * core, CPC * (core + 1))
        xc = x[:, ch]
        xz = np.zeros((B, CPC, S + 4, S, S + 4), np.float32)
        xz[:, :, 2 : S + 2, :, 2 : S + 2] = xc
        b1c = b1[ch].reshape(PAIRS, 2)
        b2c = b2[ch].reshape(PAIRS, 2)
        bias1 = np.repeat(b1c, 64, axis=1).T.copy()  # [128, PAIRS]
        bias2 = np.repeat(b2c, 64, axis=1).T.copy()
        in_maps.append(
            {
                "xz": xz,
                "xin": np.ascontiguousarray(xc),
                "bands1": _build_bands1(w1[ch]),
                "bands2": _build_bands2(w2[ch]),
                "bandsg": bandsg,
                "bias1": bias1,
                "bias2": bias2,
                "biasg": biasg,
                "zpad": zpad,
            }
        )
    return in_maps


def run(inputs, trace=False, trace_cores=None):
    """Run on 8 cores. Returns (out [2,64,64,64,64] f32, BassKernelResults)."""
    nc = _build_program()
    in_maps = _prepare_in_maps(**inputs)
    res = bass_utils.run_bass_kernel_spmd(
        nc,
        in_maps,
        core_ids=list(range(N_CORES)),
        trace=trace,
        trace_cores=trace_cores,
    )
    out = np.empty((B, C, S, S, S), np.float32)
    for core in range(N_CORES):
        out[:, CPC * core : CPC * (core + 1)] = res.results[core]["out"]
    return out, res


def kernel(x, w1, b1, w2, b2, ws, bs):
    out, _ = run(dict(x=x, w1=w1, b1=b1, w2=w2, b2=b2, ws=ws, bs=bs))
    return out


# revision 15
# speedup vs baseline: 1.0766x; 1.0766x over previous
"""Trainium2 Bass kernel for nn_DLK_35218731827409 (dense_cnn LKA-style block).

Reference computation (per batch, 64 channels, 64^3 volume):
    att1 = depthwise_conv3d(x, w1 5x5x5, pad 2) + b1
    att2 = depthwise_conv3d(att1, w2 7x7x7, dil 3, pad 9) + b2
    avg/max pooling over the 128 channels of concat(att1, att2)
    gate = sigmoid(conv3d(pooled, ws 2->2ch 7x7x7, pad 3) + bs)
    out  = att1*gate0 + att2*gate1 + x

Sharding: channels (64 -> 8 per core, both batches on every core). Depthwise
convs are channel-independent -> no halo, no redundant compute. The
cross-channel mean/max pooling is the only global step: per-core partial
sum/max + AllReduce(add) + AllReduce(max) over the 8 cores. Every core then
computes the (small) gate conv for the full volume and combines its own
channels.

Conv mapping on the TensorEngine: partitions = (2 channels x 64 H rows),
contraction along H via a banded Toeplitz lhsT (built host-side from the
depthwise weights), one PSUM-accumulated float32r matmul (N=512) per
(kd, kw) tap over (D, W) columns.
"""
import sys
import types

import numpy as np


def _install_ntff_hook():
    # Provide the antenv.axon_hooks module this image lacks so that
    # run_bass_kernel_spmd(trace=True) can reach the NTFF profiler
    # (documented degraded path in trn_agent_boot/trn_boot.py).
    if "antenv.axon_hooks" in sys.modules:
        return
    try:
        from trn_agent_boot.trn_boot import _ntff_profile_via_ctypes

        hook = _ntff_profile_via_ctypes("/opt/axon/libaxon_pjrt.so")
    except Exception:
        hook = None
    mod = types.ModuleType("antenv.axon_hooks")
    mod._hook = hook
    mod.get_axon_ntff_profile_hook = lambda: mod._hook
    mod.set_axon_ntff_profile_hook = lambda h: setattr(mod, "_hook", h)
    try:
        import antenv

        sys.modules["antenv.axon_hooks"] = mod
        antenv.axon_hooks = mod
    except Exception:
        pass


_install_ntff_hook()

import concourse.bacc as bacc
import concourse.bass_utils as bass_utils
import concourse.mybir as mybir
import concourse.tile as tile

dt = mybir.dt
AF = mybir.ActivationFunctionType
ALU = mybir.AluOpType

B, C, S = 2, 64, 64
N_CORES = 8
CPC = C // N_CORES  # 8 channels per core
PAIRS = CPC // 2  # 4 channel pairs per core
NEG_INF = -1.0e30


# ---------------------------------------------------------------- host prep
def _build_bands1(w1c):
    """w1c: [8, 1, 5, 5, 5] -> [PAIRS, 128(k), 25(t), 128(m)] f32."""
    out = np.zeros((PAIRS, 128, 25, 128), np.float32)
    ho = np.arange(64)
    for p in range(PAIRS):
        for c2 in range(2):
            c = 2 * p + c2
            for kd in range(5):
                for kw in range(5):
                    for kh in range(5):
                        hi = ho + kh - 2
                        m = (hi >= 0) & (hi < 64)
                        out[p, c2 * 64 + hi[m], kd * 5 + kw, c2 * 64 + ho[m]] = w1c[
                            c, 0, kd, kh, kw
                        ]
    return out


def _build_bands2(w2c):
    out = np.zeros((PAIRS, 128, 49, 128), np.float32)
    ho = np.arange(64)
    for p in range(PAIRS):
        for c2 in range(2):
            c = 2 * p + c2
            for kd in range(7):
                for kw in range(7):
                    for kh in range(7):
                        hi = ho + 3 * kh - 9
                        m = (hi >= 0) & (hi < 64)
                        out[p, c2 * 64 + hi[m], kd * 7 + kw, c2 * 64 + ho[m]] = w2c[
                            c, 0, kd, kh, kw
                        ]
    return out


def _build_bandsg(ws):
    """ws: [2, 2, 7, 7, 7]; fold mean 1/128 into ci=0. -> [128, 49, 128]."""
    wsx = np.array(ws, np.float32).copy()
    wsx[:, 0] /= 128.0
    out = np.zeros((128, 49, 128), np.float32)
    ho = np.arange(64)
    for ci in range(2):
        for co in range(2):
            for kd in range(7):
                for kw in range(7):
                    for kh in range(7):
                        hi = ho + kh - 3
                        m = (hi >= 0) & (hi < 64)
                        out[ci * 64 + hi[m], kd * 7 + kw, co * 64 + ho[m]] = wsx[
                            co, ci, kd, kh, kw
                        ]
    return out


# ---------------------------------------------------------------- program
_CACHE = {}


def _build_program():
    if "nc" in _CACHE:
        return _CACHE["nc"]
    f32, f32r = dt.float32, dt.float32r
    nc = bacc.Bacc(
        "TRN2", target_bir_lowering=False, debug=False, num_devices=N_CORES
    )
    xz = nc.dram_tensor("xz", [B, CPC, S + 4, S, S + 4], f32r, kind="ExternalInput").ap()
    xin = nc.dram_tensor("xin", [B, CPC, S, S, S], f32, kind="ExternalInput").ap()
    bd1_d = nc.dram_tensor("bands1", [PAIRS, 128, 25, 128], f32r, kind="ExternalInput").ap()
    bd2_d = nc.dram_tensor("bands2", [PAIRS, 128, 49, 128], f32r, kind="ExternalInput").ap()
    bdg_d = nc.dram_tensor("bandsg", [128, 49, 128], f32r, kind="ExternalInput").ap()
    b1_d = nc.dram_tensor("bias1", [128, PAIRS], f32, kind="ExternalInput").ap()
    b2_d = nc.dram_tensor("bias2", [128, PAIRS], f32, kind="ExternalInput").ap()
    bg_d = nc.dram_tensor("biasg", [128, 1], f32, kind="ExternalInput").ap()
    zp = nc.dram_tensor("zpad", [128, 64, S + 18], f32r, kind="ExternalInput").ap()
    out_d = nc.dram_tensor("out", [B, CPC, S, S, S], f32, kind="ExternalOutput").ap()

    with tile.TileContext(nc) as tc:
        with (
            tc.tile_pool(name="const", bufs=1) as constp,
            tc.tile_pool(name="dram", bufs=1, space="DRAM") as dram,
        ):
            bias1_t = constp.tile([128, PAIRS], f32)
            bias2_t = constp.tile([128, PAIRS], f32)
            biasg_t = constp.tile([128, 1], f32)
            nc.sync.dma_start(bias1_t[:], b1_d[:])
            nc.sync.dma_start(bias2_t[:], b2_d[:])
            nc.sync.dma_start(biasg_t[:], bg_d[:])

            att1_sp = dram.tile([B, PAIRS, 128, S, S], f32)
            att2_sp = dram.tile([B, PAIRS, 128, S, S], f32)
            ps_in = dram.tile([B, 64, S, S], f32)
            pm_in = dram.tile([B, 64, S, S], f32)
            ps_out = [
                dram.tile([64, S, S], f32, addr_space="Shared", name=f"ps_out{i}")
                for i in range(B)
            ]
            pm_out = [
                dram.tile([64, S, S], f32, addr_space="Shared", name=f"pm_out{i}")
                for i in range(B)
            ]

            # ---------------- conv phase ----------------
            with (
                tc.tile_pool(name="csb", bufs=1) as csb,
                tc.tile_pool(name="cps", bufs=4, space="PSUM") as cps,
            ):
                for b in range(B):
                    acc_s = csb.tile([128, S, S], f32, tag="acc_s")
                    acc_m = csb.tile([128, S, S], f32, tag="acc_m")
                    nc.vector.memset(acc_s[:], 0.0)
                    nc.vector.memset(acc_m[:], NEG_INF)
                    for p in range(PAIRS):
                        xt = csb.tile([128, S + 4, S + 4], f32r, tag="xt", bufs=2)
                        for c2 in range(2):
                            nc.sync.dma_start(
                                xt[64 * c2 : 64 * c2 + 64],
                                xz[b, 2 * p + c2].transpose([1, 0, 2]),
                            )
                        bd1 = csb.tile([128, 25, 128], f32r, tag="bd1", bufs=2)
                        nc.sync.dma_start(bd1[:], bd1_d[p])
                        bd2 = csb.tile([128, 49, 128], f32r, tag="bd2", bufs=1)
                        nc.sync.dma_start(bd2[:], bd2_d[p])

                        att1 = csb.tile([128, S + 18, S + 18], f32r, tag="att1")
                        # zero the halo border (interior is fully overwritten).
                        # memset lacks f32r ucode, so DMA from a zeros input.
                        nc.sync.dma_start(att1[:, 0:9, :], zp[:, 0:9, :])
                        nc.sync.dma_start(att1[:, S + 9 : S + 18, :], zp[:, 0:9, :])
                        nc.sync.dma_start(att1[:, 9 : S + 9, 0:9], zp[:, :, 0:9])
                        nc.sync.dma_start(
                            att1[:, 9 : S + 9, S + 9 : S + 18], zp[:, :, 0:9]
                        )

                        # conv1: 8 groups x 25 taps
                        for g in range(8):
                            ps = cps.tile([128, 8, 64], f32, tag="ps")
                            for kd in range(5):
                                for kw in range(5):
                                    t = kd * 5 + kw
                                    nc.tensor.matmul(
                                        ps[:],
                                        bd1[:, t, :],
                                        xt[:, 8 * g + kd : 8 * g + kd + 8, kw : kw + 64],
                                        start=(t == 0),
                                        stop=(t == 24),
                                    )
                            nc.scalar.activation(
                                att1[:, 9 + 8 * g : 17 + 8 * g, 9 : S + 9],
                                ps[:],
                                AF.Identity,
                                bias=bias1_t[:, p : p + 1],
                            )
                        # read the f32r interior as plain f32 bits
                        att1_int = att1[:, 9 : S + 9, 9 : S + 9].bitcast(f32)
                        nc.sync.dma_start(att1_sp[b, p], att1_int)
                        nc.vector.tensor_tensor(acc_s[:], acc_s[:], att1_int, ALU.add)
                        nc.vector.tensor_tensor(acc_m[:], acc_m[:], att1_int, ALU.max)

                        att2 = csb.tile([128, S, S], f32, tag="att2")
                        # conv2: 8 groups x 49 taps (dilation 3)
                        for g in range(8):
                            ps2 = cps.tile([128, 8, 64], f32, tag="ps")
                            for kd in range(7):
                                for kw in range(7):
                                    t = kd * 7 + kw
                                    nc.tensor.matmul(
                                        ps2[:],
                                        bd2[:, t, :],
                                        att1[
                                            :,
                                            8 * g + 3 * kd : 8 * g + 3 * kd + 8,
                                            3 * kw : 3 * kw + 64,
                                        ],
                                        start=(t == 0),
                                        stop=(t == 48),
                                    )
                            nc.scalar.activation(
                                att2[:, 8 * g : 8 * g + 8, :],
                                ps2[:],
                                AF.Identity,
                                bias=bias2_t[:, p : p + 1],
                            )
                        nc.sync.dma_start(att2_sp[b, p], att2[:])
                        nc.vector.tensor_tensor(acc_s[:], acc_s[:], att2[:], ALU.add)
                        nc.vector.tensor_tensor(acc_m[:], acc_m[:], att2[:], ALU.max)

                    # fold the two channel halves (cross-partition move via
                    # SBUF->SBUF DMA, then DVE combine) and stage for the
                    # AllReduce.
                    tmp_s = csb.tile([64, S, S], f32, tag="tmp_s")
                    tmp_m = csb.tile([64, S, S], f32, tag="tmp_m")
                    nc.sync.dma_start(tmp_s[:], acc_s[64:128])
                    nc.sync.dma_start(tmp_m[:], acc_m[64:128])
                    nc.vector.tensor_tensor(tmp_s[:], tmp_s[:], acc_s[0:64], ALU.add)
                    nc.vector.tensor_tensor(tmp_m[:], tmp_m[:], acc_m[0:64], ALU.max)
                    nc.sync.dma_start(ps_in[b], tmp_s[:])
                    nc.sync.dma_start(pm_in[b], tmp_m[:])

                    # per-batch AllReduce: batch 0's collective overlaps
                    # batch 1's convs (collectives run on TOPSP/SDMA).
                    nc.gpsimd.collective_compute(
                        "AllReduce",
                        ALU.add,
                        replica_groups=[list(range(N_CORES))],
                        ins=[ps_in[b]],
                        outs=[ps_out[b][:]],
                    )
                    nc.gpsimd.collective_compute(
                        "AllReduce",
                        ALU.max,
                        replica_groups=[list(range(N_CORES))],
                        ins=[pm_in[b]],
                        outs=[pm_out[b][:]],
                    )

            # ---------------- gate + final combine ----------------
            with (
                tc.tile_pool(name="gsb", bufs=1) as gsb,
                tc.tile_pool(name="gps", bufs=4, space="PSUM") as gps,
            ):
                bdg = gsb.tile([128, 49, 128], f32r, tag="bdg")
                nc.sync.dma_start(bdg[:], bdg_d[:])
                for b in range(B):
                    # pooled, in (ci, h) x (d, w) layout with zero borders.
                    # Loaded as f32 in d-chunks, then ACT-copied so the
                    # f32r-consuming gate matmul sees an f32r producer.
                    pt = gsb.tile([128, S + 6, S + 6], f32r, tag="pt")
                    nc.sync.dma_start(pt[:, 0:3, :], zp[:, 0:3, : S + 6])
                    nc.sync.dma_start(pt[:, S + 3 : S + 6, :], zp[:, 0:3, : S + 6])
                    nc.sync.dma_start(pt[:, 3 : S + 3, 0:3], zp[:, :, 0:3])
                    nc.sync.dma_start(pt[:, 3 : S + 3, S + 3 : S + 6], zp[:, :, 0:3])
                    for dchunk in range(2):
                        pt0 = gsb.tile([128, 32, S], f32, tag="pt0", bufs=2)
                        d0 = 32 * dchunk
                        nc.sync.dma_start(pt0[0:64], ps_out[b][:, d0 : d0 + 32, :])
                        nc.sync.dma_start(pt0[64:128], pm_out[b][:, d0 : d0 + 32, :])
                        nc.scalar.activation(
                            pt[:, 3 + d0 : 3 + d0 + 32, 3 : S + 3], pt0[:], AF.Copy
                        )

                    gA = gsb.tile([128, S, S], f32, tag="gA")
                    gB = gsb.tile([128, S, S], f32, tag="gB")
                    for g in range(8):
                        psg = gps.tile([128, 8, 64], f32, tag="psg")
                        for kd in range(7):
                            for kw in range(7):
                                t = kd * 7 + kw
                                nc.tensor.matmul(
                                    psg[:],
                                    bdg[:, t, :],
                                    pt[:, 8 * g + kd : 8 * g + kd + 8, kw : kw + 64],
                                    start=(t == 0),
                                    stop=(t == 48),
                                )
                        nc.scalar.activation(
                            gA[0:64, 8 * g : 8 * g + 8, :],
                            psg[0:64],
                            AF.Sigmoid,
                            bias=biasg_t[0:64, 0:1],
                        )
                        nc.scalar.activation(
                            gB[64:128, 8 * g : 8 * g + 8, :],
                            psg[64:128],
                            AF.Sigmoid,
                            bias=biasg_t[64:128, 0:1],
                        )
                    # replicate gate planes to the other channel half
                    # (cross-partition -> SBUF->SBUF DMA)
                    nc.sync.dma_start(gA[64:128], gA[0:64])
                    nc.sync.dma_start(gB[0:64], gB[64:128])

                    for p in range(PAIRS):
                        a1f = gsb.tile([128, S, S], f32, tag="a1f", bufs=2)
                        a2f = gsb.tile([128, S, S], f32, tag="a2f", bufs=2)
                        xf = gsb.tile([128, S, S], f32, tag="xf", bufs=1)
                        nc.sync.dma_start(a1f[:], att1_sp[b, p])
                        nc.sync.dma_start(a2f[:], att2_sp[b, p])
                        for c2 in range(2):
                            nc.sync.dma_start(
                                xf[64 * c2 : 64 * c2 + 64],
                                xin[b, 2 * p + c2].transpose([1, 0, 2]),
                            )
                        ot = gsb.tile([128, S, S], f32, tag="ot", bufs=2)
                        nc.vector.tensor_tensor(a1f[:], a1f[:], gA[:], ALU.mult)
                        nc.vector.tensor_tensor(a2f[:], a2f[:], gB[:], ALU.mult)
                        nc.vector.tensor_tensor(ot[:], a1f[:], a2f[:], ALU.add)
                        nc.vector.tensor_tensor(ot[:], ot[:], xf[:], ALU.add)
                        for c2 in range(2):
                            nc.sync.dma_start(
                                out_d[b, 2 * p + c2].transpose([1, 0, 2]),
                                ot[64 * c2 : 64 * c2 + 64],
                            )

    nc.compile()
    _CACHE["nc"] = nc
    return nc


# ---------------------------------------------------------------- runner
def _prepare_in_maps(x, w1, b1, w2, b2, ws, bs):
    x = np.ascontiguousarray(np.asarray(x, np.float32))
    w1 = np.asarray(w1, np.float32)
    b1 = np.asarray(b1, np.float32)
    w2 = np.asarray(w2, np.float32)
    b2 = np.asarray(b2, np.float32)
    ws = np.asarray(ws, np.float32)
    bs = np.asarray(bs, np.float32)

    bandsg = _build_bandsg(ws)
    biasg = np.repeat(bs, 64).reshape(128, 1).astype(np.float32)
    zpad = np.zeros((128, 64, S + 18), np.float32)

    in_maps = []
    for core in range(N_CORES):
        ch = slice(CPC * core, CPC * (core + 1))
        xc = x[:, ch]
        xz = np.zeros((B, CPC, S + 4, S, S + 4), np.float32)
        xz[:, :, 2 : S + 2, :, 2 : S + 2] = xc
        b1c = b1[ch].reshape(PAIRS, 2)
        b2c = b2[ch].reshape(PAIRS, 2)
        bias1 = np.repeat(b1c, 64, axis=1).T.copy()  # [128, PAIRS]
        bias2 = np.repeat(b2c, 64, axis=1).T.copy()
        in_maps.append(
            {
                "xz": xz,
                "xin": np.ascontiguousarray(xc),
                "bands1": _build_bands1(w1[ch]),
                "bands2": _build_bands2(w2[ch]),
                "bandsg": bandsg,
                "bias1": bias1,
                "bias2": bias2,
                "biasg": biasg,
                "zpad": zpad,
            }
        )
    return in_maps


def run(inputs, trace=False, trace_cores=None):
    """Run on 8 cores. Returns (out [2,64,64,64,64] f32, BassKernelResults)."""
    nc = _build_program()
    in_maps = _prepare_in_maps(**inputs)
    res = bass_utils.run_bass_kernel_spmd(
        nc,
        in_maps,
        core_ids=list(range(N_CORES)),
        trace=trace,
        trace_cores=trace_cores,
    )
    out = np.empty((B, C, S, S, S), np.float32)
    for core in range(N_CORES):
        out[:, CPC * core : CPC * (core + 1)] = res.results[core]["out"]
    return out, res


def kernel(x, w1, b1, w2, b2, ws, bs):
    out, _ = run(dict(x=x, w1=w1, b1=b1, w2=w2, b2=b2, ws=ws, bs=bs))
    return out
